# revision 26
# baseline (speedup 1.0000x reference)
"""Trainium2 Bass kernel for nn_MaxGraphConv (gnn_message_passing).

Reference computation (per batch element, all f32):
  xn   = L2-normalize(x^T along C)                       # (N, C)
  d2   = |xn_i - xn_j|^2 via Gram matrix, self excluded
  idx  = 16 nearest neighbors per point (smallest d2)
  md_c = max_k |xn_ic - xn_jc| over the 16 neighbors      # (N, C)
  feat = interleave(xn, md) -> (2C, N); y = W @ feat + b
  y    = BatchNorm(training stats over (B, N)) ; out = gelu_exact(y)

Sharding: data-parallel over B across 8 cores (2 batches/core); conv/BN
params replicated; BN statistics all-reduced (4KB) on device.

Device algorithm per batch (optimized vs the f32 baseline):
  * fp16 datapath for xn / scores / gather / maxdiff / conv operands
    (PE fp16 matmul = 1 cyc/row vs f32's 4; DVE 16-bit = 2x; gather
    traffic halved). BN stats + affine + gelu stay f32.
  * Since xn rows are unit-norm, d2 = 2 - 2*G: the Gram matrix G alone
    orders neighbors. score = G evicted PSUM->fp16 on ACT; no column
    norms broadcast / fused subtract needed.
  * top-16 via DVE InstMax/InstMaxIndex(u16)/InstMatchReplace (8 + 8).
  * 16 neighbor rows per point gathered with per-k indirect DMAs
    ([128,1] offsets -- multi-offset indirect DMA is broken in the
    SWDGE ucode; payloads overlap).
  * md from min/max trees over the 16 gathered rows (fp16 DVE).
  * conv as W_even @ xn + W_odd @ md (W pre-split+transposed fp16 on
    host), so no physical channel interleave is needed.
  * BN: per-channel sum/sumsq -> 4KB AllReduce -> affine+gelu on ACT.
  * conv bias b cancels exactly in training-mode BN (y+b shifts the
    mean by b) so it is accepted and ignored.
"""

import sys

if "/opt/trn_rl_repo" not in sys.path:
    sys.path.insert(0, "/opt/trn_rl_repo")

import numpy as np

import concourse.bacc as bacc
import concourse.mybir as mybir
import concourse.tile as tile
from concourse import bass
from concourse.alu_op_type import AluOpType
from concourse.bass import IndirectOffsetOnAxis
from concourse.bass_utils import run_bass_kernel_spmd
from concourse.masks import make_identity

F32 = mybir.dt.float32
F32R = mybir.dt.float32r
F16 = mybir.dt.float16
U16 = mybir.dt.uint16
I16 = mybir.dt.int16
U32 = mybir.dt.uint32
AF = mybir.ActivationFunctionType
AX = None  # set lazily (bass_rust.AxisListType.X)

N_CORES = 8
B, C, N = 16, 256, 1024
B_LOC = B // N_CORES          # 2 batches per core
OUT = 2 * C                   # 512
K_G = 16
BN_EPS = 1e-5
BIG = np.float32(30000.0)     # fp16-safe self-exclusion offset
NB = N // 128                 # 8 row blocks per batch
CT = C // 128                 # 2 channel tiles
OT = OUT // 128               # 4 out-channel tiles
NH = N // 512                 # 2 free-dim halves for matmul


def build_kernel(use_gelu=True):
    import bass_rust

    global AX
    AX = bass_rust.AxisListType.X

    nc = bacc.Bacc("TRN2", target_bir_lowering=False, debug=False)

    x_in = nc.dram_tensor("x", [B_LOC, C, N], F32, kind="ExternalInput")
    wev_in = nc.dram_tensor("wev", [C, OUT], F16, kind="ExternalInput")
    wod_in = nc.dram_tensor("wod", [C, OUT], F16, kind="ExternalInput")
    gamma_in = nc.dram_tensor("gamma4", [128, OT], F32, kind="ExternalInput")
    beta_in = nc.dram_tensor("beta4", [128, OT], F32, kind="ExternalInput")
    out_dram = nc.dram_tensor("out", [B_LOC, OUT, N], F32, kind="ExternalOutput")

    # gather sources (offset-0 requirement for indirect DMA src)
    xn_rows = [nc.dram_tensor(f"xn_rows{bi}", [N, C], F16) for bi in range(B_LOC)]
    idx_scr = nc.dram_tensor("idx_scr", [B_LOC, NB, 16, 128], I16)
    stats_in = nc.dram_tensor("stats_in", [128, 2 * OT], F32)
    stats_out = nc.dram_tensor("stats_out", [128, 2 * OT], F32)

    from contextlib import ExitStack

    with tile.TileContext(nc) as tc, ExitStack() as ctx:
        ep = ctx.enter_context
        constp = ep(tc.tile_pool(name="const", bufs=1))
        wpool = ep(tc.tile_pool(name="wpool", bufs=CT))
        xload = ep(tc.tile_pool(name="xload", bufs=2 * CT))
        sqp = ep(tc.tile_pool(name="sqp", bufs=2))
        ysqp = ep(tc.tile_pool(name="ysqp", bufs=2))
        rowp = ep(tc.tile_pool(name="rowp", bufs=4))
        bcp = ep(tc.tile_pool(name="bcp", bufs=2))
        xnp = ep(tc.tile_pool(name="xnp", bufs=2 * CT))
        xnncp = ep(tc.tile_pool(name="xnnc", bufs=2 * NB))
        scorep = ep(tc.tile_pool(name="score", bufs=2))
        idxp = ep(tc.tile_pool(name="idxp", bufs=6))
        nbrp = ep(tc.tile_pool(name="nbrp", bufs=4))
        treep = ep(tc.tile_pool(name="treep", bufs=3))
        mdncp = ep(tc.tile_pool(name="mdnc", bufs=2))
        mdcnp = ep(tc.tile_pool(name="mdcn", bufs=2 * CT))
        ypool = ep(tc.tile_pool(name="ypool", bufs=2 * OT))
        statp = ep(tc.tile_pool(name="statp", bufs=1))
        ps_row = ep(tc.tile_pool(name="ps_row", bufs=1, space="PSUM"))
        ps_bc = ep(tc.tile_pool(name="ps_bc", bufs=1, space="PSUM"))
        ps_tp = ep(tc.tile_pool(name="ps_tp", bufs=1, space="PSUM"))
        ps_mm = ep(tc.tile_pool(name="ps_mm", bufs=4, space="PSUM"))

        # ---- constants ----
        ident16 = constp.tile([128, 128], F16)
        make_identity(nc, ident16[:])
        ident32 = constp.tile([128, 128], F32)
        make_identity(nc, ident32[:])
        big_i = constp.tile([128, 128], F32)
        nc.vector.tensor_scalar_mul(big_i[:], ident16[:], float(BIG))
        ones_col = constp.tile([128, 1], F32)
        nc.vector.memset(ones_col[:], 1.0)
        ones_row32 = constp.tile([1, 128], F32)
        nc.vector.memset(ones_row32[:], 1.0)

        # prefetch the gelu activation table out of the critical tail
        warm = constp.tile([1, 8], F32)
        nc.scalar.activation(warm[:], big_i[0:1, 0:8], AF.Gelu)

        # ---- replicated weights / BN params (emitted after the first
        # x loads so they don't delay the head chain on the sync queue) ----
        wev = []
        wod = []
        gamma4 = constp.tile([128, OT], F32)
        beta4 = constp.tile([128, OT], F32)

        def load_params():
            for ct in range(CT):
                t = wpool.tile([128, OUT], F16, tag="wev")
                nc.sync.dma_start(
                    out=t[:], in_=wev_in[ct * 128:(ct + 1) * 128, :]
                )
                wev.append(t)
                t = wpool.tile([128, OUT], F16, tag="wod")
                nc.sync.dma_start(
                    out=t[:], in_=wod_in[ct * 128:(ct + 1) * 128, :]
                )
                wod.append(t)
            nc.sync.dma_start(out=gamma4[:], in_=gamma_in[:, :])
            nc.sync.dma_start(out=beta4[:], in_=beta_in[:, :])

        # per-channel partial sums of y and y^2: col = ot*4 + bi*2 + h
        part_s1 = statp.tile([128, OT * B_LOC * NH], F32)
        part_s2 = statp.tile([128, OT * B_LOC * NH], F32)

        y_tiles = {}  # (bi, ot) -> tile (128, N) f32

        def head_a(bi):
            st = {"bi": bi}
            # load x (C, N) f32
            x_ct = []
            for ct in range(CT):
                t = xload.tile([128, N], F32, tag="x")
                nc.sync.dma_start(
                    out=t[:], in_=x_in[bi, ct * 128:(ct + 1) * 128, :]
                )
                x_ct.append(t)
            if bi == 0:
                load_params()

            # column norms: s_raw[m] = sum_c x[c,m]^2 via ones-matmul
            xsq_ct = []
            for ct in range(CT):
                t = sqp.tile([128, N], F32, tag="xsq")
                nc.scalar.activation(t[:], x_ct[ct][:], AF.Square)
                xsq_ct.append(t)

            rnorm_row = rowp.tile([1, N], F32, tag="rnorm_row")
            for h in range(NH):
                hs = slice(h * 512, (h + 1) * 512)
                ps = ps_row.tile([1, 512], F32, tag="srow")
                for ct in range(CT):
                    nc.tensor.matmul(
                        out=ps[:],
                        lhsT=ones_col[:],
                        rhs=xsq_ct[ct][:, hs],
                        start=(ct == 0),
                        stop=(ct == CT - 1),
                    )
                srt = rowp.tile([1, 512], F32, tag="srt")
                nc.scalar.activation(srt[:], ps[:], AF.Sqrt)
                nc.vector.reciprocal(rnorm_row[:, hs], srt[:])

            # broadcast rnorm to 128 partitions via K=1 PE matmul
            rnorm_bc = bcp.tile([128, N], F32, tag="rnorm_bc")
            for h in range(NH):
                hs = slice(h * 512, (h + 1) * 512)
                ps = ps_bc.tile([128, 512], F32, tag="bc")
                nc.tensor.matmul(
                    out=ps[:],
                    lhsT=ones_row32[:],
                    rhs=rnorm_row[:, hs],
                    start=True,
                    stop=True,
                )
                nc.scalar.copy(rnorm_bc[:, hs], ps[:])

            # normalize: xn32 f32 (Gram + transposes), fp16 cast (conv rhs)
            xn32_ct = []
            xn_ct = []
            for ct in range(CT):
                t32 = xnp.tile([128, N], F32, tag="xn32")
                nc.vector.tensor_tensor(
                    t32[:], x_ct[ct][:], rnorm_bc[:], op=AluOpType.mult
                )
                xn32_ct.append(t32)
            for ct in range(CT):
                t = xnp.tile([128, N], F16, tag="xn")
                nc.scalar.copy(t[:], xn32_ct[ct][:])
                xn_ct.append(t)

            st.update(x_ct=x_ct, xn32_ct=xn32_ct, xn_ct=xn_ct)
            return st

        def head_b(st):
            bi = st["bi"]
            xn32_ct = st["xn32_ct"]
            # transpose xn32 -> (N, C) fp16 rows (cast in the PSUM evict)
            xn_nc = []
            for nb in range(NB):
                t = xnncp.tile([128, C], F16, tag="xn_nc")
                for ct in range(CT):
                    ps = ps_tp.tile([128, 128], F32, tag="tp")
                    nc.tensor.transpose(
                        out=ps[:],
                        in_=xn32_ct[ct][:, nb * 128:(nb + 1) * 128],
                        identity=ident32[:],
                    )
                    nc.scalar.copy(t[:, ct * 128:(ct + 1) * 128], ps[:])
                nc.sync.dma_start(
                    out=xn_rows[bi][nb * 128:(nb + 1) * 128, :], in_=t[:]
                )
                xn_nc.append(t)

            md_cn = []
            for ct in range(CT):
                md_cn.append(
                    mdcnp.tile([128, N], F16, tag="md_cn", name=f"md_cn{bi}_{ct}")
                )
            st.update(xn_nc=xn_nc, md_cn=md_cn)
            return st

        def topk_part(st, rb):
            bi = st["bi"]
            xn32_ct = st["xn32_ct"]
            rbs = slice(rb * 128, (rb + 1) * 128)
            score = scorep.tile([128, N], F32, tag="score")
            for h in range(NH):
                hs = slice(h * 512, (h + 1) * 512)
                ps = ps_mm.tile([128, 512], F32, tag="mm")
                for ct in range(CT):
                    nc.tensor.matmul(
                        out=ps[:],
                        lhsT=xn32_ct[ct][:, rbs],
                        rhs=xn32_ct[ct][:, hs],
                        start=(ct == 0),
                        stop=(ct == CT - 1),
                    )
                # score = G (unit-norm rows: larger G == nearer); ACT evict
                nc.scalar.copy(score[:, hs], ps[:])
            # self-exclusion: score[p, rb*128+p] -= BIG
            nc.vector.tensor_tensor(
                score[:, rbs], score[:, rbs], big_i[:], op=AluOpType.subtract
            )

            # top-16: 8 + 8 via max8/max_index(u16)/match_replace
            idx16h = idxp.tile([128, K_G], U16, tag="idxh")
            m8 = idxp.tile([128, 8], F32, tag="m8")
            nc.vector.max(out=m8[:], in_=score[:])
            nc.vector.max_index(
                out=idx16h[:, 0:8], in_max=m8[:], in_values=score[:]
            )
            nc.vector.match_replace(
                out=score[:],
                in_to_replace=m8[:],
                in_values=score[:],
                imm_value=float(-BIG),
            )
            m8b = idxp.tile([128, 8], F32, tag="m8b")
            nc.vector.max(out=m8b[:], in_=score[:])
            nc.vector.max_index(
                out=idx16h[:, 8:16], in_max=m8b[:], in_values=score[:]
            )
            idx32 = idxp.tile([128, K_G], U32, tag="idx32")
            nc.vector.tensor_copy(idx32[:], idx16h[:])
            return idx32

        def gather_part(st, rb, idx32):
            bi = st["bi"]
            # gather 16 neighbor rows (per-k indirect DMA, fp16 rows).
            # ~8ns/descriptor of Q7 SWDGE time is the hard floor here;
            # dma_gather costs the same Q7 time but pipelines worse.
            nbr = nbrp.tile([128, K_G, C], F16, tag="nbr")
            for s in range(K_G):
                nc.gpsimd.indirect_dma_start(
                    out=nbr[:, s, :],
                    out_offset=None,
                    in_=xn_rows[bi][:],
                    in_offset=IndirectOffsetOnAxis(
                        ap=idx32[:, s:s + 1], axis=0
                    ),
                )
            return nbr

        def topk_gather(st, rb):
            return gather_part(st, rb, topk_part(st, rb))

        def trees_md(st, rb, nbr):
            xn_nc = st["xn_nc"]
            md_cn = st["md_cn"]
            rbs = slice(rb * 128, (rb + 1) * 128)
            tmax = treep.tile([128, K_G // 2, C], F16, tag="tmax")
            tmin = treep.tile([128, K_G // 2, C], F16, tag="tmin")
            nc.vector.tensor_tensor(
                tmax[:], nbr[:, 0:8, :], nbr[:, 8:16, :], op=AluOpType.max
            )
            nc.vector.tensor_tensor(
                tmin[:], nbr[:, 0:8, :], nbr[:, 8:16, :], op=AluOpType.min
            )
            w_ = 4
            while w_ >= 1:
                nc.vector.tensor_tensor(
                    tmax[:, 0:w_, :],
                    tmax[:, 0:w_, :],
                    tmax[:, w_:2 * w_, :],
                    op=AluOpType.max,
                )
                nc.vector.tensor_tensor(
                    tmin[:, 0:w_, :],
                    tmin[:, 0:w_, :],
                    tmin[:, w_:2 * w_, :],
                    op=AluOpType.min,
                )
                w_ //= 2

            # md = max(xn - min, max - xn)
            md_nc = mdncp.tile([128, C], F16, tag="md_nc")
            d1 = mdncp.tile([128, C], F16, tag="d1")
            nc.vector.tensor_tensor(
                d1[:], xn_nc[rb][:], tmin[:, 0, :], op=AluOpType.subtract
            )
            nc.vector.tensor_tensor(
                md_nc[:], tmax[:, 0, :], xn_nc[rb][:], op=AluOpType.subtract
            )
            nc.vector.tensor_tensor(
                md_nc[:], md_nc[:], d1[:], op=AluOpType.max
            )

            # transpose md block into (C, N) fp16 tiles
            for ct in range(CT):
                ps = ps_tp.tile([128, 128], F16, tag="tp16")
                nc.tensor.transpose(
                    out=ps[:],
                    in_=md_nc[:, ct * 128:(ct + 1) * 128],
                    identity=ident16[:],
                )
                nc.scalar.copy(md_cn[ct][:, rbs], ps[:])

        def conv(st):
            bi = st["bi"]
            xn_ct = st["xn_ct"]
            md_cn = st["md_cn"]
            for ot in range(OT):
                ots = slice(ot * 128, (ot + 1) * 128)
                yt = ypool.tile([128, N], F32, tag="y")
                y_tiles[(bi, ot)] = yt
                for h in range(NH):
                    hs = slice(h * 512, (h + 1) * 512)
                    ps = ps_mm.tile([128, 512], F32, tag="mm")
                    for ct in range(CT):
                        nc.tensor.matmul(
                            out=ps[:],
                            lhsT=wev[ct][:, ots],
                            rhs=xn_ct[ct][:, hs],
                            start=(ct == 0),
                            stop=False,
                        )
                    for ct in range(CT):
                        nc.tensor.matmul(
                            out=ps[:],
                            lhsT=wod[ct][:, ots],
                            rhs=md_cn[ct][:, hs],
                            start=False,
                            stop=(ct == CT - 1),
                        )
                    # move PSUM->SBUF on ACT with fused per-channel sum
                    col = ot * (B_LOC * NH) + bi * NH + h
                    nc.scalar.activation(
                        yt[:, hs],
                        ps[:],
                        AF.Copy,
                        accum_out=part_s1[:, col:col + 1],
                    )
                    # sumsq via ACT Square with fused per-channel sum
                    sq_scr = ysqp.tile([128, 512], F32, tag="ysq")
                    nc.scalar.activation(
                        sq_scr[:],
                        yt[:, hs],
                        AF.Square,
                        accum_out=part_s2[:, col:col + 1],
                    )

        # batch-0 head (its first Gram/topk chains overlap its own
        # transposes), batch-1 head under batch-0 gathers, interleaved row
        # blocks; convs at the very end so they don't block the last top-ks.
        from collections import deque

        states = [None, None]
        states[0] = head_a(0)
        order = [(0, 0), (0, 1)]
        tail0 = [(0, r) for r in range(2, NB)]
        all1 = [(1, r) for r in range(NB)]
        for i in range(len(all1)):
            order.append(all1[i])
            if i < len(tail0):
                order.append(tail0[i])
        head_b(states[0])
        pending = {0: deque(), 1: deque()}
        for n_emitted, (bi, rb) in enumerate(order):
            if states[bi] is None:
                states[bi] = head_a(bi)
                head_b(states[bi])
            nbr = topk_gather(states[bi], rb)
            pending[bi].append((rb, nbr))
            if len(pending[bi]) > 3:
                trees_md(states[bi], *pending[bi].popleft())
            if n_emitted >= len(order) - 2:
                # pre-drain OLD pending groups (gathers >=2 bursts back,
                # guaranteed complete) so the tail drain shrinks without
                # stalling the DVE queue behind in-flight gathers
                if len(pending[0]) > 1:
                    trees_md(states[0], *pending[0].popleft())
                if len(pending[1]) > 2:
                    trees_md(states[1], *pending[1].popleft())
        for bi in range(B_LOC):
            while pending[bi]:
                trees_md(states[bi], *pending[bi].popleft())
            conv(states[bi])

        # ---- BN stats: reduce partials, all-reduce across cores ----
        stats_sb = statp.tile([128, 2 * OT], F32)
        nc.vector.tensor_reduce(
            stats_sb[:, 0:OT],
            part_s1[:].rearrange("p (o q) -> p o q", q=B_LOC * NH),
            axis=AX,
            op=AluOpType.add,
        )
        nc.vector.tensor_reduce(
            stats_sb[:, OT:2 * OT],
            part_s2[:].rearrange("p (o q) -> p o q", q=B_LOC * NH),
            axis=AX,
            op=AluOpType.add,
        )
        nc.sync.dma_start(out=stats_in[:, :], in_=stats_sb[:])
        nc.gpsimd.collective_compute(
            "AllReduce",
            AluOpType.add,
            replica_groups=[list(range(N_CORES))],
            ins=[stats_in.ap().opt()],
            outs=[stats_out.ap().opt()],
        )
        stats_red = statp.tile([128, 2 * OT], F32)
        nc.sync.dma_start(out=stats_red[:], in_=stats_out[:, :])

        # mean/var/affine (per channel; channel c = partition p, col ot)
        inv_cnt = 1.0 / float(B * N)
        mean4 = statp.tile([128, OT], F32)
        nc.vector.tensor_scalar_mul(mean4[:], stats_red[:, 0:OT], inv_cnt)
        msq = statp.tile([128, OT], F32)
        nc.vector.tensor_tensor(msq[:], mean4[:], mean4[:], op=AluOpType.mult)
        var4 = statp.tile([128, OT], F32)
        nc.vector.scalar_tensor_tensor(
            out=var4[:],
            in0=stats_red[:, OT:2 * OT],
            scalar=inv_cnt,
            in1=msq[:],
            op0=AluOpType.mult,
            op1=AluOpType.subtract,
        )
        nc.vector.tensor_scalar_add(var4[:], var4[:], float(BN_EPS))
        std4 = statp.tile([128, OT], F32)
        nc.scalar.activation(std4[:], var4[:], AF.Sqrt)
        rstd4 = statp.tile([128, OT], F32)
        nc.vector.reciprocal(rstd4[:], std4[:])
        a4 = statp.tile([128, OT], F32)
        nc.vector.tensor_tensor(a4[:], gamma4[:], rstd4[:], op=AluOpType.mult)
        b4 = statp.tile([128, OT], F32)
        nc.vector.scalar_tensor_tensor(
            out=b4[:],
            in0=mean4[:],
            scalar=-1.0,
            in1=a4[:],
            op0=AluOpType.mult,
            op1=AluOpType.mult,
        )
        nc.vector.tensor_tensor(b4[:], b4[:], beta4[:], op=AluOpType.add)

        # ---- fused BN + exact gelu on ACT, then store ----
        for bi in range(B_LOC):
            for ot in range(OT):
                yt = y_tiles[(bi, ot)]
                for h in range(NH):
                    hs = slice(h * 512, (h + 1) * 512)
                    nc.scalar.activation(
                        yt[:, hs],
                        yt[:, hs],
                        AF.Gelu if use_gelu else AF.Copy,
                        bias=b4[:, ot:ot + 1] if use_gelu else 0.0,
                        scale=a4[:, ot:ot + 1],
                    )
                nc.sync.dma_start(
                    out=out_dram[bi, ot * 128:(ot + 1) * 128, :], in_=yt[:]
                )

    nc.compile()
    return nc


_NC_CACHE = None


def _get_nc():
    global _NC_CACHE
    if _NC_CACHE is None:
        _NC_CACHE = build_kernel()
    return _NC_CACHE


def _prep_shared(w, gamma, beta):
    w = np.asarray(w, np.float32)
    wev = np.ascontiguousarray(w[:, 0::2].T).astype(np.float16)  # (C, OUT)
    wod = np.ascontiguousarray(w[:, 1::2].T).astype(np.float16)
    gamma4 = np.ascontiguousarray(
        np.asarray(gamma, np.float32).reshape(OT, 128).T
    )
    beta4 = np.ascontiguousarray(np.asarray(beta, np.float32).reshape(OT, 128).T)
    return wev, wod, gamma4, beta4


def kernel(x, w, b, gamma, beta):
    x = np.ascontiguousarray(np.asarray(x, np.float32))
    assert x.shape == (B, C, N), x.shape
    wev, wod, gamma4, beta4 = _prep_shared(w, gamma, beta)
    # b cancels exactly in training-mode BN (see module docstring).
    nc = _get_nc()
    in_maps = [
        {
            "x": np.ascontiguousarray(x[c * B_LOC:(c + 1) * B_LOC]),
            "wev": wev,
            "wod": wod,
            "gamma4": gamma4,
            "beta4": beta4,
        }
        for c in range(N_CORES)
    ]
    res = run_bass_kernel_spmd(nc, in_maps, core_ids=list(range(N_CORES)))
    out = np.concatenate([res.results[c]["out"] for c in range(N_CORES)], axis=0)
    return out[..., None].astype(np.float32)


# revision 27
# speedup vs baseline: 1.0350x; 1.0350x over previous
"""Trainium2 Bass kernel for nn_MaxGraphConv (gnn_message_passing).

Reference computation (per batch element, all f32):
  xn   = L2-normalize(x^T along C)                       # (N, C)
  d2   = |xn_i - xn_j|^2 via Gram matrix, self excluded
  idx  = 16 nearest neighbors per point (smallest d2)
  md_c = max_k |xn_ic - xn_jc| over the 16 neighbors      # (N, C)
  feat = interleave(xn, md) -> (2C, N); y = W @ feat + b
  y    = BatchNorm(training stats over (B, N)) ; out = gelu_exact(y)

Sharding: data-parallel over B across 8 cores (2 batches/core); conv/BN
params replicated; BN statistics all-reduced (4KB) on device.

Device algorithm per batch (optimized vs the f32 baseline):
  * fp16 datapath for xn / scores / gather / maxdiff / conv operands
    (PE fp16 matmul = 1 cyc/row vs f32's 4; DVE 16-bit = 2x; gather
    traffic halved). BN stats + affine + gelu stay f32.
  * Since xn rows are unit-norm, d2 = 2 - 2*G: the Gram matrix G alone
    orders neighbors. score = G evicted PSUM->fp16 on ACT; no column
    norms broadcast / fused subtract needed.
  * top-16 via DVE InstMax/InstMaxIndex(u16)/InstMatchReplace (8 + 8).
  * 16 neighbor rows per point gathered with per-k indirect DMAs
    ([128,1] offsets -- multi-offset indirect DMA is broken in the
    SWDGE ucode; payloads overlap).
  * md from min/max trees over the 16 gathered rows (fp16 DVE).
  * conv as W_even @ xn + W_odd @ md (W pre-split+transposed fp16 on
    host), so no physical channel interleave is needed.
  * BN: per-channel sum/sumsq -> 4KB AllReduce -> affine+gelu on ACT.
  * conv bias b cancels exactly in training-mode BN (y+b shifts the
    mean by b) so it is accepted and ignored.
"""

import sys

if "/opt/trn_rl_repo" not in sys.path:
    sys.path.insert(0, "/opt/trn_rl_repo")

import numpy as np

import concourse.bacc as bacc
import concourse.mybir as mybir
import concourse.tile as tile
from concourse import bass
from concourse.alu_op_type import AluOpType
from concourse.bass import IndirectOffsetOnAxis
from concourse.bass_utils import run_bass_kernel_spmd
from concourse.masks import make_identity

F32 = mybir.dt.float32
F32R = mybir.dt.float32r
F16 = mybir.dt.float16
U16 = mybir.dt.uint16
I16 = mybir.dt.int16
U32 = mybir.dt.uint32
AF = mybir.ActivationFunctionType
AX = None  # set lazily (bass_rust.AxisListType.X)

N_CORES = 8
B, C, N = 16, 256, 1024
B_LOC = B // N_CORES          # 2 batches per core
OUT = 2 * C                   # 512
K_G = 16
BN_EPS = 1e-5
BIG = np.float32(30000.0)     # fp16-safe self-exclusion offset
NB = N // 128                 # 8 row blocks per batch
CT = C // 128                 # 2 channel tiles
OT = OUT // 128               # 4 out-channel tiles
NH = N // 512                 # 2 free-dim halves for matmul


def build_kernel(use_gelu=True):
    import bass_rust

    global AX
    AX = bass_rust.AxisListType.X

    nc = bacc.Bacc("TRN2", target_bir_lowering=False, debug=False)

    x_in = nc.dram_tensor("x", [B_LOC, C, N], F32, kind="ExternalInput")
    wev_in = nc.dram_tensor("wev", [C, OUT], F16, kind="ExternalInput")
    wod_in = nc.dram_tensor("wod", [C, OUT], F16, kind="ExternalInput")
    gamma_in = nc.dram_tensor("gamma4", [128, OT], F32, kind="ExternalInput")
    beta_in = nc.dram_tensor("beta4", [128, OT], F32, kind="ExternalInput")
    out_dram = nc.dram_tensor("out", [B_LOC, OUT, N], F32, kind="ExternalOutput")

    # gather sources (offset-0 requirement for indirect DMA src)
    xn_rows = [nc.dram_tensor(f"xn_rows{bi}", [N, C], F16) for bi in range(B_LOC)]
    idx_scr = nc.dram_tensor("idx_scr", [B_LOC, NB, 16, 128], I16)
    stats_in = nc.dram_tensor("stats_in", [128, 2 * OT], F32)
    stats_out = nc.dram_tensor("stats_out", [128, 2 * OT], F32)

    from contextlib import ExitStack

    with tile.TileContext(nc) as tc, ExitStack() as ctx:
        ep = ctx.enter_context
        constp = ep(tc.tile_pool(name="const", bufs=1))
        wpool = ep(tc.tile_pool(name="wpool", bufs=CT))
        xload = ep(tc.tile_pool(name="xload", bufs=2 * CT))
        sqp = ep(tc.tile_pool(name="sqp", bufs=2))
        ysqp = ep(tc.tile_pool(name="ysqp", bufs=2))
        rowp = ep(tc.tile_pool(name="rowp", bufs=4))
        bcp = ep(tc.tile_pool(name="bcp", bufs=2))
        xnp = ep(tc.tile_pool(name="xnp", bufs=2 * CT))
        xnncp = ep(tc.tile_pool(name="xnnc", bufs=2 * NB))
        scorep = ep(tc.tile_pool(name="score", bufs=2))
        idxp = ep(tc.tile_pool(name="idxp", bufs=6))
        nbrp = ep(tc.tile_pool(name="nbrp", bufs=4))
        treep = ep(tc.tile_pool(name="treep", bufs=3))
        mdncp = ep(tc.tile_pool(name="mdnc", bufs=2))
        mdcnp = ep(tc.tile_pool(name="mdcn", bufs=2 * CT))
        ypool = ep(tc.tile_pool(name="ypool", bufs=2 * OT))
        statp = ep(tc.tile_pool(name="statp", bufs=1))
        ps_row = ep(tc.tile_pool(name="ps_row", bufs=1, space="PSUM"))
        ps_bc = ep(tc.tile_pool(name="ps_bc", bufs=1, space="PSUM"))
        ps_tp = ep(tc.tile_pool(name="ps_tp", bufs=1, space="PSUM"))
        ps_mm = ep(tc.tile_pool(name="ps_mm", bufs=4, space="PSUM"))

        # ---- constants ----
        ident16 = constp.tile([128, 128], F16)
        make_identity(nc, ident16[:])
        ident32 = constp.tile([128, 128], F32)
        make_identity(nc, ident32[:])
        big_i = constp.tile([128, 128], F32)
        nc.vector.tensor_scalar_mul(big_i[:], ident16[:], float(BIG))
        ones_col = constp.tile([128, 1], F32)
        nc.vector.memset(ones_col[:], 1.0)
        ones_row32 = constp.tile([1, 128], F32)
        nc.vector.memset(ones_row32[:], 1.0)

        # prefetch the gelu activation table out of the critical tail
        warm = constp.tile([1, 8], F32)
        nc.scalar.activation(warm[:], big_i[0:1, 0:8], AF.Gelu)

        # ---- replicated weights / BN params (emitted after the first
        # x loads so they don't delay the head chain on the sync queue) ----
        wev = []
        wod = []
        gamma4 = constp.tile([128, OT], F32)
        beta4 = constp.tile([128, OT], F32)

        def load_params():
            for ct in range(CT):
                t = wpool.tile([128, OUT], F16, tag="wev")
                nc.sync.dma_start(
                    out=t[:], in_=wev_in[ct * 128:(ct + 1) * 128, :]
                )
                wev.append(t)
                t = wpool.tile([128, OUT], F16, tag="wod")
                nc.sync.dma_start(
                    out=t[:], in_=wod_in[ct * 128:(ct + 1) * 128, :]
                )
                wod.append(t)
            nc.sync.dma_start(out=gamma4[:], in_=gamma_in[:, :])
            nc.sync.dma_start(out=beta4[:], in_=beta_in[:, :])

        # per-channel partial sums of y and y^2: col = ot*4 + bi*2 + h
        part_s1 = statp.tile([128, OT * B_LOC * NH], F32)
        part_s2 = statp.tile([128, OT * B_LOC * NH], F32)

        y_tiles = {}  # (bi, ot) -> tile (128, N) f32

        def head_a(bi):
            st = {"bi": bi}
            # load x (C, N) f32
            x_ct = []
            for ct in range(CT):
                t = xload.tile([128, N], F32, tag="x")
                nc.sync.dma_start(
                    out=t[:], in_=x_in[bi, ct * 128:(ct + 1) * 128, :]
                )
                x_ct.append(t)
            if bi == 0:
                load_params()

            # column norms: s_raw[m] = sum_c x[c,m]^2 via ones-matmul
            xsq_ct = []
            for ct in range(CT):
                t = sqp.tile([128, N], F32, tag="xsq")
                nc.scalar.activation(t[:], x_ct[ct][:], AF.Square)
                xsq_ct.append(t)

            rnorm_row = rowp.tile([1, N], F32, tag="rnorm_row")
            for h in range(NH):
                hs = slice(h * 512, (h + 1) * 512)
                ps = ps_row.tile([1, 512], F32, tag="srow")
                for ct in range(CT):
                    nc.tensor.matmul(
                        out=ps[:],
                        lhsT=ones_col[:],
                        rhs=xsq_ct[ct][:, hs],
                        start=(ct == 0),
                        stop=(ct == CT - 1),
                    )
                srt = rowp.tile([1, 512], F32, tag="srt")
                nc.scalar.activation(srt[:], ps[:], AF.Sqrt)
                nc.vector.reciprocal(rnorm_row[:, hs], srt[:])

            # broadcast rnorm to 128 partitions via K=1 PE matmul
            rnorm_bc = bcp.tile([128, N], F32, tag="rnorm_bc")
            for h in range(NH):
                hs = slice(h * 512, (h + 1) * 512)
                ps = ps_bc.tile([128, 512], F32, tag="bc")
                nc.tensor.matmul(
                    out=ps[:],
                    lhsT=ones_row32[:],
                    rhs=rnorm_row[:, hs],
                    start=True,
                    stop=True,
                )
                nc.scalar.copy(rnorm_bc[:, hs], ps[:])

            # normalize: xn32 f32 (Gram + transposes), fp16 cast (conv rhs)
            xn32_ct = []
            xn_ct = []
            for ct in range(CT):
                t32 = xnp.tile([128, N], F32, tag="xn32")
                nc.vector.tensor_tensor(
                    t32[:], x_ct[ct][:], rnorm_bc[:], op=AluOpType.mult
                )
                xn32_ct.append(t32)
            for ct in range(CT):
                t = xnp.tile([128, N], F16, tag="xn")
                nc.scalar.copy(t[:], xn32_ct[ct][:])
                xn_ct.append(t)

            st.update(x_ct=x_ct, xn32_ct=xn32_ct, xn_ct=xn_ct)
            return st

        def head_b(st):
            bi = st["bi"]
            xn32_ct = st["xn32_ct"]
            # transpose xn32 -> (N, C) fp16 rows (cast in the PSUM evict)
            xn_nc = []
            for nb in range(NB):
                t = xnncp.tile([128, C], F16, tag="xn_nc")
                for ct in range(CT):
                    ps = ps_tp.tile([128, 128], F32, tag="tp")
                    nc.tensor.transpose(
                        out=ps[:],
                        in_=xn32_ct[ct][:, nb * 128:(nb + 1) * 128],
                        identity=ident32[:],
                    )
                    nc.scalar.copy(t[:, ct * 128:(ct + 1) * 128], ps[:])
                nc.sync.dma_start(
                    out=xn_rows[bi][nb * 128:(nb + 1) * 128, :], in_=t[:]
                )
                xn_nc.append(t)

            md_cn = []
            for ct in range(CT):
                md_cn.append(
                    mdcnp.tile([128, N], F16, tag="md_cn", name=f"md_cn{bi}_{ct}")
                )
            st.update(xn_nc=xn_nc, md_cn=md_cn)
            return st

        def topk_part(st, rb):
            bi = st["bi"]
            xn32_ct = st["xn32_ct"]
            rbs = slice(rb * 128, (rb + 1) * 128)
            score = scorep.tile([128, N], F32, tag="score")
            for h in range(NH):
                hs = slice(h * 512, (h + 1) * 512)
                ps = ps_mm.tile([128, 512], F32, tag="mm")
                for ct in range(CT):
                    nc.tensor.matmul(
                        out=ps[:],
                        lhsT=xn32_ct[ct][:, rbs],
                        rhs=xn32_ct[ct][:, hs],
                        start=(ct == 0),
                        stop=(ct == CT - 1),
                    )
                # score = G (unit-norm rows: larger G == nearer); ACT evict
                nc.scalar.copy(score[:, hs], ps[:])
            # self-exclusion: score[p, rb*128+p] -= BIG
            nc.vector.tensor_tensor(
                score[:, rbs], score[:, rbs], big_i[:], op=AluOpType.subtract
            )

            # top-16: 8 + 8 via max8/max_index(u16)/match_replace
            idx16h = idxp.tile([128, K_G], U16, tag="idxh")
            m8 = idxp.tile([128, 8], F32, tag="m8")
            nc.vector.max(out=m8[:], in_=score[:])
            nc.vector.max_index(
                out=idx16h[:, 0:8], in_max=m8[:], in_values=score[:]
            )
            nc.vector.match_replace(
                out=score[:],
                in_to_replace=m8[:],
                in_values=score[:],
                imm_value=float(-BIG),
            )
            m8b = idxp.tile([128, 8], F32, tag="m8b")
            nc.vector.max(out=m8b[:], in_=score[:])
            nc.vector.max_index(
                out=idx16h[:, 8:16], in_max=m8b[:], in_values=score[:]
            )
            idx32 = idxp.tile([128, K_G], U32, tag="idx32")
            nc.vector.tensor_copy(idx32[:], idx16h[:])
            return idx32

        def gather_part(st, rb, idx32):
            bi = st["bi"]
            # gather 16 neighbor rows (per-k indirect DMA, fp16 rows).
            # ~8ns/descriptor of Q7 SWDGE time is the hard floor here;
            # dma_gather costs the same Q7 time but pipelines worse.
            nbr = nbrp.tile([128, K_G, C], F16, tag="nbr")
            for s in range(K_G):
                nc.gpsimd.indirect_dma_start(
                    out=nbr[:, s, :],
                    out_offset=None,
                    in_=xn_rows[bi][:],
                    in_offset=IndirectOffsetOnAxis(
                        ap=idx32[:, s:s + 1], axis=0
                    ),
                )
            return nbr

        def topk_gather(st, rb):
            return gather_part(st, rb, topk_part(st, rb))

        def trees_md(st, rb, nbr):
            xn_nc = st["xn_nc"]
            md_cn = st["md_cn"]
            rbs = slice(rb * 128, (rb + 1) * 128)
            tmax = treep.tile([128, K_G // 2, C], F16, tag="tmax")
            tmin = treep.tile([128, K_G // 2, C], F16, tag="tmin")
            nc.vector.tensor_tensor(
                tmax[:], nbr[:, 0:8, :], nbr[:, 8:16, :], op=AluOpType.max
            )
            nc.vector.tensor_tensor(
                tmin[:], nbr[:, 0:8, :], nbr[:, 8:16, :], op=AluOpType.min
            )
            w_ = 4
            while w_ >= 1:
                nc.vector.tensor_tensor(
                    tmax[:, 0:w_, :],
                    tmax[:, 0:w_, :],
                    tmax[:, w_:2 * w_, :],
                    op=AluOpType.max,
                )
                nc.vector.tensor_tensor(
                    tmin[:, 0:w_, :],
                    tmin[:, 0:w_, :],
                    tmin[:, w_:2 * w_, :],
                    op=AluOpType.min,
                )
                w_ //= 2

            # md = max(xn - min, max - xn)
            md_nc = mdncp.tile([128, C], F16, tag="md_nc")
            d1 = mdncp.tile([128, C], F16, tag="d1")
            nc.vector.tensor_tensor(
                d1[:], xn_nc[rb][:], tmin[:, 0, :], op=AluOpType.subtract
            )
            nc.vector.tensor_tensor(
                md_nc[:], tmax[:, 0, :], xn_nc[rb][:], op=AluOpType.subtract
            )
            nc.vector.tensor_tensor(
                md_nc[:], md_nc[:], d1[:], op=AluOpType.max
            )

            # transpose md block into (C, N) fp16 tiles
            for ct in range(CT):
                ps = ps_tp.tile([128, 128], F16, tag="tp16")
                nc.tensor.transpose(
                    out=ps[:],
                    in_=md_nc[:, ct * 128:(ct + 1) * 128],
                    identity=ident16[:],
                )
                nc.scalar.copy(md_cn[ct][:, rbs], ps[:])

        def conv(st):
            bi = st["bi"]
            xn_ct = st["xn_ct"]
            md_cn = st["md_cn"]
            for ot in range(OT):
                ots = slice(ot * 128, (ot + 1) * 128)
                yt = ypool.tile([128, N], F32, tag="y")
                y_tiles[(bi, ot)] = yt
                for h in range(NH):
                    hs = slice(h * 512, (h + 1) * 512)
                    ps = ps_mm.tile([128, 512], F32, tag="mm")
                    for ct in range(CT):
                        nc.tensor.matmul(
                            out=ps[:],
                            lhsT=wev[ct][:, ots],
                            rhs=xn_ct[ct][:, hs],
                            start=(ct == 0),
                            stop=False,
                        )
                    for ct in range(CT):
                        nc.tensor.matmul(
                            out=ps[:],
                            lhsT=wod[ct][:, ots],
                            rhs=md_cn[ct][:, hs],
                            start=False,
                            stop=(ct == CT - 1),
                        )
                    # move PSUM->SBUF on ACT with fused per-channel sum
                    col = ot * (B_LOC * NH) + bi * NH + h
                    nc.scalar.activation(
                        yt[:, hs],
                        ps[:],
                        AF.Copy,
                        accum_out=part_s1[:, col:col + 1],
                    )
                    # sumsq via ACT Square with fused per-channel sum
                    sq_scr = ysqp.tile([128, 512], F32, tag="ysq")
                    nc.scalar.activation(
                        sq_scr[:],
                        yt[:, hs],
                        AF.Square,
                        accum_out=part_s2[:, col:col + 1],
                    )

        # batch-0 head (its first Gram/topk chains overlap its own
        # transposes), batch-1 head under batch-0 gathers, interleaved row
        # blocks; convs at the very end so they don't block the last top-ks.
        from collections import deque

        states = [None, None]
        states[0] = head_a(0)
        order = [(0, 0), (0, 1)]
        tail0 = [(0, r) for r in range(2, NB)]
        all1 = [(1, r) for r in range(NB)]
        for i in range(len(all1)):
            order.append(all1[i])
            if i < len(tail0):
                order.append(tail0[i])
        head_b(states[0])
        pending = {0: deque(), 1: deque()}
        for bi, rb in order:
            if states[bi] is None:
                states[bi] = head_a(bi)
                head_b(states[bi])
            nbr = topk_gather(states[bi], rb)
            pending[bi].append((rb, nbr))
            if len(pending[bi]) > 3:
                trees_md(states[bi], *pending[bi].popleft())
        for bi in range(B_LOC):
            while pending[bi]:
                trees_md(states[bi], *pending[bi].popleft())
            conv(states[bi])

        # ---- BN stats: reduce partials, all-reduce across cores ----
        stats_sb = statp.tile([128, 2 * OT], F32)
        nc.vector.tensor_reduce(
            stats_sb[:, 0:OT],
            part_s1[:].rearrange("p (o q) -> p o q", q=B_LOC * NH),
            axis=AX,
            op=AluOpType.add,
        )
        nc.vector.tensor_reduce(
            stats_sb[:, OT:2 * OT],
            part_s2[:].rearrange("p (o q) -> p o q", q=B_LOC * NH),
            axis=AX,
            op=AluOpType.add,
        )
        nc.sync.dma_start(out=stats_in[:, :], in_=stats_sb[:])
        nc.gpsimd.collective_compute(
            "AllReduce",
            AluOpType.add,
            replica_groups=[list(range(N_CORES))],
            ins=[stats_in.ap().opt()],
            outs=[stats_out.ap().opt()],
        )
        stats_red = statp.tile([128, 2 * OT], F32)
        nc.sync.dma_start(out=stats_red[:], in_=stats_out[:, :])

        # mean/var/affine (per channel; channel c = partition p, col ot)
        inv_cnt = 1.0 / float(B * N)
        mean4 = statp.tile([128, OT], F32)
        nc.vector.tensor_scalar_mul(mean4[:], stats_red[:, 0:OT], inv_cnt)
        msq = statp.tile([128, OT], F32)
        nc.vector.tensor_tensor(msq[:], mean4[:], mean4[:], op=AluOpType.mult)
        var4 = statp.tile([128, OT], F32)
        nc.vector.scalar_tensor_tensor(
            out=var4[:],
            in0=stats_red[:, OT:2 * OT],
            scalar=inv_cnt,
            in1=msq[:],
            op0=AluOpType.mult,
            op1=AluOpType.subtract,
        )
        nc.vector.tensor_scalar_add(var4[:], var4[:], float(BN_EPS))
        std4 = statp.tile([128, OT], F32)
        nc.scalar.activation(std4[:], var4[:], AF.Sqrt)
        rstd4 = statp.tile([128, OT], F32)
        nc.vector.reciprocal(rstd4[:], std4[:])
        a4 = statp.tile([128, OT], F32)
        nc.vector.tensor_tensor(a4[:], gamma4[:], rstd4[:], op=AluOpType.mult)
        b4 = statp.tile([128, OT], F32)
        nc.vector.scalar_tensor_tensor(
            out=b4[:],
            in0=mean4[:],
            scalar=-1.0,
            in1=a4[:],
            op0=AluOpType.mult,
            op1=AluOpType.mult,
        )
        nc.vector.tensor_tensor(b4[:], b4[:], beta4[:], op=AluOpType.add)

        # ---- fused BN + exact gelu on ACT, then store ----
        for bi in range(B_LOC):
            for ot in range(OT):
                yt = y_tiles[(bi, ot)]
                for h in range(NH):
                    hs = slice(h * 512, (h + 1) * 512)
                    nc.scalar.activation(
                        yt[:, hs],
                        yt[:, hs],
                        AF.Gelu if use_gelu else AF.Copy,
                        bias=b4[:, ot:ot + 1] if use_gelu else 0.0,
                        scale=a4[:, ot:ot + 1],
                    )
                nc.sync.dma_start(
                    out=out_dram[bi, ot * 128:(ot + 1) * 128, :], in_=yt[:]
                )

    nc.compile()
    return nc


_NC_CACHE = None


def _get_nc():
    global _NC_CACHE
    if _NC_CACHE is None:
        _NC_CACHE = build_kernel()
    return _NC_CACHE


def _prep_shared(w, gamma, beta):
    w = np.asarray(w, np.float32)
    wev = np.ascontiguousarray(w[:, 0::2].T).astype(np.float16)  # (C, OUT)
    wod = np.ascontiguousarray(w[:, 1::2].T).astype(np.float16)
    gamma4 = np.ascontiguousarray(
        np.asarray(gamma, np.float32).reshape(OT, 128).T
    )
    beta4 = np.ascontiguousarray(np.asarray(beta, np.float32).reshape(OT, 128).T)
    return wev, wod, gamma4, beta4


def kernel(x, w, b, gamma, beta):
    x = np.ascontiguousarray(np.asarray(x, np.float32))
    assert x.shape == (B, C, N), x.shape
    wev, wod, gamma4, beta4 = _prep_shared(w, gamma, beta)
    # b cancels exactly in training-mode BN (see module docstring).
    nc = _get_nc()
    in_maps = [
        {
            "x": np.ascontiguousarray(x[c * B_LOC:(c + 1) * B_LOC]),
            "wev": wev,
            "wod": wod,
            "gamma4": gamma4,
            "beta4": beta4,
        }
        for c in range(N_CORES)
    ]
    res = run_bass_kernel_spmd(nc, in_maps, core_ids=list(range(N_CORES)))
    out = np.concatenate([res.results[c]["out"] for c in range(N_CORES)], axis=0)
    return out[..., None].astype(np.float32)


# revision 29
# speedup vs baseline: 1.0641x; 1.0281x over previous
"""Trainium2 Bass kernel for nn_MaxGraphConv (gnn_message_passing).

Reference computation (per batch element, all f32):
  xn   = L2-normalize(x^T along C)                       # (N, C)
  d2   = |xn_i - xn_j|^2 via Gram matrix, self excluded
  idx  = 16 nearest neighbors per point (smallest d2)
  md_c = max_k |xn_ic - xn_jc| over the 16 neighbors      # (N, C)
  feat = interleave(xn, md) -> (2C, N); y = W @ feat + b
  y    = BatchNorm(training stats over (B, N)) ; out = gelu_exact(y)

Sharding: data-parallel over B across 8 cores (2 batches/core); conv/BN
params replicated; BN statistics all-reduced (4KB) on device.

Device algorithm per batch (optimized vs the f32 baseline):
  * fp16 datapath for xn / scores / gather / maxdiff / conv operands
    (PE fp16 matmul = 1 cyc/row vs f32's 4; DVE 16-bit = 2x; gather
    traffic halved). BN stats + affine + gelu stay f32.
  * Since xn rows are unit-norm, d2 = 2 - 2*G: the Gram matrix G alone
    orders neighbors. score = G evicted PSUM->fp16 on ACT; no column
    norms broadcast / fused subtract needed.
  * top-16 via DVE InstMax/InstMaxIndex(u16)/InstMatchReplace (8 + 8).
  * 16 neighbor rows per point gathered with per-k indirect DMAs
    ([128,1] offsets -- multi-offset indirect DMA is broken in the
    SWDGE ucode; payloads overlap).
  * md from min/max trees over the 16 gathered rows (fp16 DVE).
  * conv as W_even @ xn + W_odd @ md (W pre-split+transposed fp16 on
    host), so no physical channel interleave is needed.
  * BN: per-channel sum/sumsq -> 4KB AllReduce -> affine+gelu on ACT.
  * conv bias b cancels exactly in training-mode BN (y+b shifts the
    mean by b) so it is accepted and ignored.
"""

import sys

if "/opt/trn_rl_repo" not in sys.path:
    sys.path.insert(0, "/opt/trn_rl_repo")

import numpy as np

import concourse.bacc as bacc
import concourse.mybir as mybir
import concourse.tile as tile
from concourse import bass
from concourse.alu_op_type import AluOpType
from concourse.bass import IndirectOffsetOnAxis
from concourse.bass_utils import run_bass_kernel_spmd
from concourse.masks import make_identity

F32 = mybir.dt.float32
F32R = mybir.dt.float32r
F16 = mybir.dt.float16
U16 = mybir.dt.uint16
I16 = mybir.dt.int16
U32 = mybir.dt.uint32
AF = mybir.ActivationFunctionType
AX = None  # set lazily (bass_rust.AxisListType.X)

N_CORES = 8
B, C, N = 16, 256, 1024
B_LOC = B // N_CORES          # 2 batches per core
OUT = 2 * C                   # 512
K_G = 16
BN_EPS = 1e-5
BIG = np.float32(30000.0)     # fp16-safe self-exclusion offset
NB = N // 128                 # 8 row blocks per batch
CT = C // 128                 # 2 channel tiles
OT = OUT // 128               # 4 out-channel tiles
NH = N // 512                 # 2 free-dim halves for matmul


def build_kernel(use_gelu=True):
    import bass_rust

    global AX
    AX = bass_rust.AxisListType.X

    nc = bacc.Bacc("TRN2", target_bir_lowering=False, debug=False)

    x_in = nc.dram_tensor("x", [B_LOC, C, N], F32, kind="ExternalInput")
    wev_in = nc.dram_tensor("wev", [C, OUT], F16, kind="ExternalInput")
    wod_in = nc.dram_tensor("wod", [C, OUT], F16, kind="ExternalInput")
    gamma_in = nc.dram_tensor("gamma4", [128, OT], F32, kind="ExternalInput")
    beta_in = nc.dram_tensor("beta4", [128, OT], F32, kind="ExternalInput")
    out_dram = nc.dram_tensor("out", [B_LOC, OUT, N], F32, kind="ExternalOutput")

    # gather sources (offset-0 requirement for indirect DMA src)
    xn_rows = [nc.dram_tensor(f"xn_rows{bi}", [N, C], F16) for bi in range(B_LOC)]
    idx_scr = nc.dram_tensor("idx_scr", [B_LOC, NB, 16, 128], I16)
    stats_in = nc.dram_tensor("stats_in", [128, 2 * OT], F32)
    stats_out = nc.dram_tensor("stats_out", [128, 2 * OT], F32)

    from contextlib import ExitStack

    with tile.TileContext(nc) as tc, ExitStack() as ctx:
        ep = ctx.enter_context
        constp = ep(tc.tile_pool(name="const", bufs=1))
        wpool = ep(tc.tile_pool(name="wpool", bufs=CT))
        xload = ep(tc.tile_pool(name="xload", bufs=2 * CT))
        sqp = ep(tc.tile_pool(name="sqp", bufs=2))
        ysqp = ep(tc.tile_pool(name="ysqp", bufs=2))
        rowp = ep(tc.tile_pool(name="rowp", bufs=4))
        bcp = ep(tc.tile_pool(name="bcp", bufs=2))
        xnp = ep(tc.tile_pool(name="xnp", bufs=2 * CT))
        xnncp = ep(tc.tile_pool(name="xnnc", bufs=2 * NB))
        scorep = ep(tc.tile_pool(name="score", bufs=2))
        idxp = ep(tc.tile_pool(name="idxp", bufs=6))
        nbrp = ep(tc.tile_pool(name="nbrp", bufs=4))
        treep = ep(tc.tile_pool(name="treep", bufs=3))
        mdncp = ep(tc.tile_pool(name="mdnc", bufs=2))
        mdcnp = ep(tc.tile_pool(name="mdcn", bufs=2 * CT))
        ypool = ep(tc.tile_pool(name="ypool", bufs=2 * OT))
        statp = ep(tc.tile_pool(name="statp", bufs=1))
        ps_row = ep(tc.tile_pool(name="ps_row", bufs=1, space="PSUM"))
        ps_bc = ep(tc.tile_pool(name="ps_bc", bufs=1, space="PSUM"))
        ps_tp = ep(tc.tile_pool(name="ps_tp", bufs=1, space="PSUM"))
        ps_mm = ep(tc.tile_pool(name="ps_mm", bufs=4, space="PSUM"))

        # ---- constants ----
        ident16 = constp.tile([128, 128], F16)
        make_identity(nc, ident16[:])
        ident32 = constp.tile([128, 128], F32)
        make_identity(nc, ident32[:])
        big_i = constp.tile([128, 128], F32)
        nc.vector.tensor_scalar_mul(big_i[:], ident16[:], float(BIG))
        ones_col = constp.tile([128, 1], F32)
        nc.vector.memset(ones_col[:], 1.0)
        ones_row32 = constp.tile([1, 128], F32)
        nc.vector.memset(ones_row32[:], 1.0)

        # prefetch the gelu activation table out of the critical tail
        warm = constp.tile([1, 8], F32)
        nc.scalar.activation(warm[:], big_i[0:1, 0:8], AF.Gelu)

        # ---- replicated weights / BN params (emitted after the first
        # x loads so they don't delay the head chain on the sync queue) ----
        wev = []
        wod = []
        gamma4 = constp.tile([128, OT], F32)
        beta4 = constp.tile([128, OT], F32)

        def load_params():
            for ct in range(CT):
                t = wpool.tile([128, OUT], F16, tag="wev")
                nc.sync.dma_start(
                    out=t[:], in_=wev_in[ct * 128:(ct + 1) * 128, :]
                )
                wev.append(t)
                t = wpool.tile([128, OUT], F16, tag="wod")
                nc.sync.dma_start(
                    out=t[:], in_=wod_in[ct * 128:(ct + 1) * 128, :]
                )
                wod.append(t)
            nc.sync.dma_start(out=gamma4[:], in_=gamma_in[:, :])
            nc.sync.dma_start(out=beta4[:], in_=beta_in[:, :])

        # per-channel partial sums of y and y^2: col = ot*4 + bi*2 + h
        part_s1 = statp.tile([128, OT * B_LOC * NH], F32)
        part_s2 = statp.tile([128, OT * B_LOC * NH], F32)

        y_tiles = {}  # (bi, ot) -> tile (128, N) f32

        def head_a(bi):
            st = {"bi": bi}
            # load x (C, N) f32
            x_ct = []
            for ct in range(CT):
                t = xload.tile([128, N], F32, tag="x")
                nc.sync.dma_start(
                    out=t[:], in_=x_in[bi, ct * 128:(ct + 1) * 128, :]
                )
                x_ct.append(t)
            if bi == 0:
                load_params()

            # column norms: s_raw[m] = sum_c x[c,m]^2 via ones-matmul
            xsq_ct = []
            for ct in range(CT):
                t = sqp.tile([128, N], F32, tag="xsq")
                nc.scalar.activation(t[:], x_ct[ct][:], AF.Square)
                xsq_ct.append(t)

            rnorm_row = rowp.tile([1, N], F32, tag="rnorm_row")
            for h in range(NH):
                hs = slice(h * 512, (h + 1) * 512)
                ps = ps_row.tile([1, 512], F32, tag="srow")
                for ct in range(CT):
                    nc.tensor.matmul(
                        out=ps[:],
                        lhsT=ones_col[:],
                        rhs=xsq_ct[ct][:, hs],
                        start=(ct == 0),
                        stop=(ct == CT - 1),
                    )
                srt = rowp.tile([1, 512], F32, tag="srt")
                nc.scalar.activation(srt[:], ps[:], AF.Sqrt)
                nc.vector.reciprocal(rnorm_row[:, hs], srt[:])

            # broadcast rnorm to 128 partitions via K=1 PE matmul
            rnorm_bc = bcp.tile([128, N], F32, tag="rnorm_bc")
            for h in range(NH):
                hs = slice(h * 512, (h + 1) * 512)
                ps = ps_bc.tile([128, 512], F32, tag="bc")
                nc.tensor.matmul(
                    out=ps[:],
                    lhsT=ones_row32[:],
                    rhs=rnorm_row[:, hs],
                    start=True,
                    stop=True,
                )
                nc.scalar.copy(rnorm_bc[:, hs], ps[:])

            # normalize: xn32 f32 (Gram + transposes), fp16 cast (conv rhs)
            xn32_ct = []
            xn_ct = []
            for ct in range(CT):
                t32 = xnp.tile([128, N], F32, tag="xn32")
                nc.vector.tensor_tensor(
                    t32[:], x_ct[ct][:], rnorm_bc[:], op=AluOpType.mult
                )
                xn32_ct.append(t32)
            for ct in range(CT):
                t = xnp.tile([128, N], F16, tag="xn")
                nc.scalar.copy(t[:], xn32_ct[ct][:])
                xn_ct.append(t)

            st.update(x_ct=x_ct, xn32_ct=xn32_ct, xn_ct=xn_ct)
            return st

        def head_b(st):
            bi = st["bi"]
            xn32_ct = st["xn32_ct"]
            # transpose xn32 -> (N, C) fp16 rows (cast in the PSUM evict)
            xn_nc = []
            for nb in range(NB):
                t = xnncp.tile([128, C], F16, tag="xn_nc")
                for ct in range(CT):
                    ps = ps_tp.tile([128, 128], F32, tag="tp")
                    nc.tensor.transpose(
                        out=ps[:],
                        in_=xn32_ct[ct][:, nb * 128:(nb + 1) * 128],
                        identity=ident32[:],
                    )
                    nc.scalar.copy(t[:, ct * 128:(ct + 1) * 128], ps[:])
                nc.sync.dma_start(
                    out=xn_rows[bi][nb * 128:(nb + 1) * 128, :], in_=t[:]
                )
                xn_nc.append(t)

            md_cn = []
            for ct in range(CT):
                md_cn.append(
                    mdcnp.tile([128, N], F16, tag="md_cn", name=f"md_cn{bi}_{ct}")
                )
            st.update(xn_nc=xn_nc, md_cn=md_cn)
            return st

        def topk_part(st, rb):
            bi = st["bi"]
            xn32_ct = st["xn32_ct"]
            rbs = slice(rb * 128, (rb + 1) * 128)
            score = scorep.tile([128, N], F32, tag="score")
            for h in range(NH):
                hs = slice(h * 512, (h + 1) * 512)
                ps = ps_mm.tile([128, 512], F32, tag="mm")
                for ct in range(CT):
                    nc.tensor.matmul(
                        out=ps[:],
                        lhsT=xn32_ct[ct][:, rbs],
                        rhs=xn32_ct[ct][:, hs],
                        start=(ct == 0),
                        stop=(ct == CT - 1),
                    )
                # score = G (unit-norm rows: larger G == nearer); ACT evict
                nc.scalar.copy(score[:, hs], ps[:])
            # self-exclusion: score[p, rb*128+p] -= BIG
            nc.vector.tensor_tensor(
                score[:, rbs], score[:, rbs], big_i[:], op=AluOpType.subtract
            )

            # top-16: 8 + 8 via max8/max_index(u16)/match_replace
            idx16h = idxp.tile([128, K_G], U16, tag="idxh")
            m8 = idxp.tile([128, 8], F32, tag="m8")
            nc.vector.max(out=m8[:], in_=score[:])
            nc.vector.max_index(
                out=idx16h[:, 0:8], in_max=m8[:], in_values=score[:]
            )
            nc.vector.match_replace(
                out=score[:],
                in_to_replace=m8[:],
                in_values=score[:],
                imm_value=float(-BIG),
            )
            m8b = idxp.tile([128, 8], F32, tag="m8b")
            nc.vector.max(out=m8b[:], in_=score[:])
            nc.vector.max_index(
                out=idx16h[:, 8:16], in_max=m8b[:], in_values=score[:]
            )
            idx32 = idxp.tile([128, K_G], U32, tag="idx32")
            nc.vector.tensor_copy(idx32[:], idx16h[:])
            return idx32

        def gather_part(st, rb, idx32):
            bi = st["bi"]
            # gather 16 neighbor rows (per-k indirect DMA, fp16 rows).
            # ~8ns/descriptor of Q7 SWDGE time is the hard floor here;
            # dma_gather costs the same Q7 time but pipelines worse.
            nbr = nbrp.tile([128, K_G, C], F16, tag="nbr")
            for s in range(K_G):
                nc.gpsimd.indirect_dma_start(
                    out=nbr[:, s, :],
                    out_offset=None,
                    in_=xn_rows[bi][:],
                    in_offset=IndirectOffsetOnAxis(
                        ap=idx32[:, s:s + 1], axis=0
                    ),
                )
            return nbr

        def topk_gather(st, rb):
            return gather_part(st, rb, topk_part(st, rb))

        def trees_md(st, rb, nbr):
            xn_nc = st["xn_nc"]
            md_cn = st["md_cn"]
            rbs = slice(rb * 128, (rb + 1) * 128)
            tmax = treep.tile([128, K_G // 2, C], F16, tag="tmax")
            tmin = treep.tile([128, K_G // 2, C], F16, tag="tmin")
            nc.vector.tensor_tensor(
                tmax[:], nbr[:, 0:8, :], nbr[:, 8:16, :], op=AluOpType.max
            )
            nc.vector.tensor_tensor(
                tmin[:], nbr[:, 0:8, :], nbr[:, 8:16, :], op=AluOpType.min
            )
            w_ = 4
            while w_ >= 1:
                nc.vector.tensor_tensor(
                    tmax[:, 0:w_, :],
                    tmax[:, 0:w_, :],
                    tmax[:, w_:2 * w_, :],
                    op=AluOpType.max,
                )
                nc.vector.tensor_tensor(
                    tmin[:, 0:w_, :],
                    tmin[:, 0:w_, :],
                    tmin[:, w_:2 * w_, :],
                    op=AluOpType.min,
                )
                w_ //= 2

            # md = max(xn - min, max - xn)
            md_nc = mdncp.tile([128, C], F16, tag="md_nc")
            d1 = mdncp.tile([128, C], F16, tag="d1")
            nc.vector.tensor_tensor(
                d1[:], xn_nc[rb][:], tmin[:, 0, :], op=AluOpType.subtract
            )
            nc.vector.tensor_tensor(
                md_nc[:], tmax[:, 0, :], xn_nc[rb][:], op=AluOpType.subtract
            )
            nc.vector.tensor_tensor(
                md_nc[:], md_nc[:], d1[:], op=AluOpType.max
            )

            # transpose md block into (C, N) fp16 tiles
            for ct in range(CT):
                ps = ps_tp.tile([128, 128], F16, tag="tp16")
                nc.tensor.transpose(
                    out=ps[:],
                    in_=md_nc[:, ct * 128:(ct + 1) * 128],
                    identity=ident16[:],
                )
                nc.scalar.copy(md_cn[ct][:, rbs], ps[:])

        def conv_half(st, h):
            # conv for columns [h*512,(h+1)*512) -- only needs md_cn from
            # row blocks h*4..h*4+3, so h=0 can run under the gather stream
            bi = st["bi"]
            xn_ct = st["xn_ct"]
            md_cn = st["md_cn"]
            hs = slice(h * 512, (h + 1) * 512)
            for ot in range(OT):
                ots = slice(ot * 128, (ot + 1) * 128)
                if (bi, ot) not in y_tiles:
                    yt = ypool.tile([128, N], F32, tag="y", name=f"y{bi}_{ot}")
                    y_tiles[(bi, ot)] = yt
                yt = y_tiles[(bi, ot)]
                ps = ps_mm.tile([128, 512], F32, tag="mm")
                for ct in range(CT):
                    nc.tensor.matmul(
                        out=ps[:],
                        lhsT=wev[ct][:, ots],
                        rhs=xn_ct[ct][:, hs],
                        start=(ct == 0),
                        stop=False,
                    )
                for ct in range(CT):
                    nc.tensor.matmul(
                        out=ps[:],
                        lhsT=wod[ct][:, ots],
                        rhs=md_cn[ct][:, hs],
                        start=False,
                        stop=(ct == CT - 1),
                    )
                # move PSUM->SBUF on ACT with fused per-channel sum
                col = ot * (B_LOC * NH) + bi * NH + h
                nc.scalar.activation(
                    yt[:, hs],
                    ps[:],
                    AF.Copy,
                    accum_out=part_s1[:, col:col + 1],
                )
                # sumsq via ACT Square with fused per-channel sum
                sq_scr = ysqp.tile([128, 512], F32, tag="ysq")
                nc.scalar.activation(
                    sq_scr[:],
                    yt[:, hs],
                    AF.Square,
                    accum_out=part_s2[:, col:col + 1],
                )

        # batch-0 head (its first Gram/topk chains overlap its own
        # transposes), batch-1 head under batch-0 gathers, interleaved row
        # blocks; convs at the very end so they don't block the last top-ks.
        from collections import deque

        states = [None, None]
        states[0] = head_a(0)
        order = [(0, 0), (0, 1)]
        tail0 = [(0, r) for r in range(2, NB)]
        all1 = [(1, r) for r in range(NB)]
        for i in range(len(all1)):
            order.append(all1[i])
            if i < len(tail0):
                order.append(tail0[i])
        head_b(states[0])
        pending = {0: deque(), 1: deque()}
        trees_done = {0: 0, 1: 0}

        def pop_tree(bj):
            trees_md(states[bj], *pending[bj].popleft())
            trees_done[bj] += 1
            if trees_done[bj] == 4:
                conv_half(states[bj], 0)

        for bi, rb in order:
            if states[bi] is None:
                states[bi] = head_a(bi)
                head_b(states[bi])
            nbr = topk_gather(states[bi], rb)
            pending[bi].append((rb, nbr))
            if len(pending[bi]) > 3:
                pop_tree(bi)
        for bi in range(B_LOC):
            while pending[bi]:
                pop_tree(bi)
            conv_half(states[bi], 1)

        # ---- BN stats: reduce partials, all-reduce across cores ----
        stats_sb = statp.tile([128, 2 * OT], F32)
        nc.vector.tensor_reduce(
            stats_sb[:, 0:OT],
            part_s1[:].rearrange("p (o q) -> p o q", q=B_LOC * NH),
            axis=AX,
            op=AluOpType.add,
        )
        nc.vector.tensor_reduce(
            stats_sb[:, OT:2 * OT],
            part_s2[:].rearrange("p (o q) -> p o q", q=B_LOC * NH),
            axis=AX,
            op=AluOpType.add,
        )
        nc.sync.dma_start(out=stats_in[:, :], in_=stats_sb[:])
        nc.gpsimd.collective_compute(
            "AllReduce",
            AluOpType.add,
            replica_groups=[list(range(N_CORES))],
            ins=[stats_in.ap().opt()],
            outs=[stats_out.ap().opt()],
        )
        stats_red = statp.tile([128, 2 * OT], F32)
        nc.sync.dma_start(out=stats_red[:], in_=stats_out[:, :])

        # mean/var/affine (per channel; channel c = partition p, col ot)
        inv_cnt = 1.0 / float(B * N)
        mean4 = statp.tile([128, OT], F32)
        nc.vector.tensor_scalar_mul(mean4[:], stats_red[:, 0:OT], inv_cnt)
        msq = statp.tile([128, OT], F32)
        nc.vector.tensor_tensor(msq[:], mean4[:], mean4[:], op=AluOpType.mult)
        var4 = statp.tile([128, OT], F32)
        nc.vector.scalar_tensor_tensor(
            out=var4[:],
            in0=stats_red[:, OT:2 * OT],
            scalar=inv_cnt,
            in1=msq[:],
            op0=AluOpType.mult,
            op1=AluOpType.subtract,
        )
        nc.vector.tensor_scalar_add(var4[:], var4[:], float(BN_EPS))
        std4 = statp.tile([128, OT], F32)
        nc.scalar.activation(std4[:], var4[:], AF.Sqrt)
        rstd4 = statp.tile([128, OT], F32)
        nc.vector.reciprocal(rstd4[:], std4[:])
        a4 = statp.tile([128, OT], F32)
        nc.vector.tensor_tensor(a4[:], gamma4[:], rstd4[:], op=AluOpType.mult)
        b4 = statp.tile([128, OT], F32)
        nc.vector.scalar_tensor_tensor(
            out=b4[:],
            in0=mean4[:],
            scalar=-1.0,
            in1=a4[:],
            op0=AluOpType.mult,
            op1=AluOpType.mult,
        )
        nc.vector.tensor_tensor(b4[:], b4[:], beta4[:], op=AluOpType.add)

        # ---- fused BN + exact gelu on ACT, then store ----
        for bi in range(B_LOC):
            for ot in range(OT):
                yt = y_tiles[(bi, ot)]
                for h in range(NH):
                    hs = slice(h * 512, (h + 1) * 512)
                    nc.scalar.activation(
                        yt[:, hs],
                        yt[:, hs],
                        AF.Gelu if use_gelu else AF.Copy,
                        bias=b4[:, ot:ot + 1] if use_gelu else 0.0,
                        scale=a4[:, ot:ot + 1],
                    )
                nc.sync.dma_start(
                    out=out_dram[bi, ot * 128:(ot + 1) * 128, :], in_=yt[:]
                )

    nc.compile()
    return nc


_NC_CACHE = None


def _get_nc():
    global _NC_CACHE
    if _NC_CACHE is None:
        _NC_CACHE = build_kernel()
    return _NC_CACHE


def _prep_shared(w, gamma, beta):
    w = np.asarray(w, np.float32)
    wev = np.ascontiguousarray(w[:, 0::2].T).astype(np.float16)  # (C, OUT)
    wod = np.ascontiguousarray(w[:, 1::2].T).astype(np.float16)
    gamma4 = np.ascontiguousarray(
        np.asarray(gamma, np.float32).reshape(OT, 128).T
    )
    beta4 = np.ascontiguousarray(np.asarray(beta, np.float32).reshape(OT, 128).T)
    return wev, wod, gamma4, beta4


def kernel(x, w, b, gamma, beta):
    x = np.ascontiguousarray(np.asarray(x, np.float32))
    assert x.shape == (B, C, N), x.shape
    wev, wod, gamma4, beta4 = _prep_shared(w, gamma, beta)
    # b cancels exactly in training-mode BN (see module docstring).
    nc = _get_nc()
    in_maps = [
        {
            "x": np.ascontiguousarray(x[c * B_LOC:(c + 1) * B_LOC]),
            "wev": wev,
            "wod": wod,
            "gamma4": gamma4,
            "beta4": beta4,
        }
        for c in range(N_CORES)
    ]
    res = run_bass_kernel_spmd(nc, in_maps, core_ids=list(range(N_CORES)))
    out = np.concatenate([res.results[c]["out"] for c in range(N_CORES)], axis=0)
    return out[..., None].astype(np.float32)


# revision 30
# speedup vs baseline: 1.0648x; 1.0007x over previous
"""Trainium2 Bass kernel for nn_MaxGraphConv (gnn_message_passing).

Reference computation (per batch element, all f32):
  xn   = L2-normalize(x^T along C)                       # (N, C)
  d2   = |xn_i - xn_j|^2 via Gram matrix, self excluded
  idx  = 16 nearest neighbors per point (smallest d2)
  md_c = max_k |xn_ic - xn_jc| over the 16 neighbors      # (N, C)
  feat = interleave(xn, md) -> (2C, N); y = W @ feat + b
  y    = BatchNorm(training stats over (B, N)) ; out = gelu_exact(y)

Sharding: data-parallel over B across 8 cores (2 batches/core); conv/BN
params replicated; BN statistics all-reduced (4KB) on device.

Device algorithm per batch (optimized vs the f32 baseline):
  * fp16 datapath for xn / scores / gather / maxdiff / conv operands
    (PE fp16 matmul = 1 cyc/row vs f32's 4; DVE 16-bit = 2x; gather
    traffic halved). BN stats + affine + gelu stay f32.
  * Since xn rows are unit-norm, d2 = 2 - 2*G: the Gram matrix G alone
    orders neighbors. score = G evicted PSUM->fp16 on ACT; no column
    norms broadcast / fused subtract needed.
  * top-16 via DVE InstMax/InstMaxIndex(u16)/InstMatchReplace (8 + 8).
  * 16 neighbor rows per point gathered with per-k indirect DMAs
    ([128,1] offsets -- multi-offset indirect DMA is broken in the
    SWDGE ucode; payloads overlap).
  * md from min/max trees over the 16 gathered rows (fp16 DVE).
  * conv as W_even @ xn + W_odd @ md (W pre-split+transposed fp16 on
    host), so no physical channel interleave is needed.
  * BN: per-channel sum/sumsq -> 4KB AllReduce -> affine+gelu on ACT.
  * conv bias b cancels exactly in training-mode BN (y+b shifts the
    mean by b) so it is accepted and ignored.
"""

import sys

if "/opt/trn_rl_repo" not in sys.path:
    sys.path.insert(0, "/opt/trn_rl_repo")

import numpy as np

import concourse.bacc as bacc
import concourse.mybir as mybir
import concourse.tile as tile
from concourse import bass
from concourse.alu_op_type import AluOpType
from concourse.bass import IndirectOffsetOnAxis
from concourse.bass_utils import run_bass_kernel_spmd
from concourse.masks import make_identity

F32 = mybir.dt.float32
F32R = mybir.dt.float32r
F16 = mybir.dt.float16
U16 = mybir.dt.uint16
I16 = mybir.dt.int16
U32 = mybir.dt.uint32
AF = mybir.ActivationFunctionType
AX = None  # set lazily (bass_rust.AxisListType.X)

N_CORES = 8
B, C, N = 16, 256, 1024
B_LOC = B // N_CORES          # 2 batches per core
OUT = 2 * C                   # 512
K_G = 16
BN_EPS = 1e-5
BIG = np.float32(30000.0)     # fp16-safe self-exclusion offset
NB = N // 128                 # 8 row blocks per batch
CT = C // 128                 # 2 channel tiles
OT = OUT // 128               # 4 out-channel tiles
NH = N // 512                 # 2 free-dim halves for matmul


def build_kernel(use_gelu=True):
    import bass_rust

    global AX
    AX = bass_rust.AxisListType.X

    nc = bacc.Bacc("TRN2", target_bir_lowering=False, debug=False)

    x_in = nc.dram_tensor("x", [B_LOC, C, N], F32, kind="ExternalInput")
    wev_in = nc.dram_tensor("wev", [C, OUT], F16, kind="ExternalInput")
    wod_in = nc.dram_tensor("wod", [C, OUT], F16, kind="ExternalInput")
    gamma_in = nc.dram_tensor("gamma4", [128, OT], F32, kind="ExternalInput")
    beta_in = nc.dram_tensor("beta4", [128, OT], F32, kind="ExternalInput")
    out_dram = nc.dram_tensor("out", [B_LOC, OUT, N], F32, kind="ExternalOutput")

    # gather sources (offset-0 requirement for indirect DMA src)
    xn_rows = [nc.dram_tensor(f"xn_rows{bi}", [N, C], F16) for bi in range(B_LOC)]
    idx_scr = nc.dram_tensor("idx_scr", [B_LOC, NB, 16, 128], I16)
    stats_in = nc.dram_tensor("stats_in", [128, 2 * OT], F32)
    stats_out = nc.dram_tensor("stats_out", [128, 2 * OT], F32)

    from contextlib import ExitStack

    with tile.TileContext(nc) as tc, ExitStack() as ctx:
        ep = ctx.enter_context
        constp = ep(tc.tile_pool(name="const", bufs=1))
        wpool = ep(tc.tile_pool(name="wpool", bufs=CT))
        xload = ep(tc.tile_pool(name="xload", bufs=2 * CT))
        sqp = ep(tc.tile_pool(name="sqp", bufs=2))
        ysqp = ep(tc.tile_pool(name="ysqp", bufs=2))
        rowp = ep(tc.tile_pool(name="rowp", bufs=4))
        bcp = ep(tc.tile_pool(name="bcp", bufs=2))
        xnp = ep(tc.tile_pool(name="xnp", bufs=2 * CT))
        xnncp = ep(tc.tile_pool(name="xnnc", bufs=2 * NB))
        scorep = ep(tc.tile_pool(name="score", bufs=2))
        idxp = ep(tc.tile_pool(name="idxp", bufs=6))
        nbrp = ep(tc.tile_pool(name="nbrp", bufs=4))
        treep = ep(tc.tile_pool(name="treep", bufs=3))
        mdncp = ep(tc.tile_pool(name="mdnc", bufs=2))
        mdcnp = ep(tc.tile_pool(name="mdcn", bufs=2 * CT))
        ypool = ep(tc.tile_pool(name="ypool", bufs=2 * OT))
        statp = ep(tc.tile_pool(name="statp", bufs=1))
        ps_row = ep(tc.tile_pool(name="ps_row", bufs=1, space="PSUM"))
        ps_bc = ep(tc.tile_pool(name="ps_bc", bufs=1, space="PSUM"))
        ps_tp = ep(tc.tile_pool(name="ps_tp", bufs=1, space="PSUM"))
        ps_mm = ep(tc.tile_pool(name="ps_mm", bufs=4, space="PSUM"))

        # ---- constants ----
        ident16 = constp.tile([128, 128], F16)
        make_identity(nc, ident16[:])
        ident32 = constp.tile([128, 128], F32)
        make_identity(nc, ident32[:])
        big_i = constp.tile([128, 128], F32)
        nc.vector.tensor_scalar_mul(big_i[:], ident16[:], float(BIG))
        ones_col = constp.tile([128, 1], F32)
        nc.vector.memset(ones_col[:], 1.0)
        ones_row32 = constp.tile([1, 128], F32)
        nc.vector.memset(ones_row32[:], 1.0)

        # prefetch the gelu activation table out of the critical tail
        warm = constp.tile([1, 8], F32)
        nc.scalar.activation(warm[:], big_i[0:1, 0:8], AF.Gelu)

        # ---- replicated weights / BN params (emitted after the first
        # x loads so they don't delay the head chain on the sync queue) ----
        wev = []
        wod = []
        gamma4 = constp.tile([128, OT], F32)
        beta4 = constp.tile([128, OT], F32)

        def load_params():
            for ct in range(CT):
                t = wpool.tile([128, OUT], F16, tag="wev")
                nc.sync.dma_start(
                    out=t[:], in_=wev_in[ct * 128:(ct + 1) * 128, :]
                )
                wev.append(t)
                t = wpool.tile([128, OUT], F16, tag="wod")
                nc.sync.dma_start(
                    out=t[:], in_=wod_in[ct * 128:(ct + 1) * 128, :]
                )
                wod.append(t)
            nc.sync.dma_start(out=gamma4[:], in_=gamma_in[:, :])
            nc.sync.dma_start(out=beta4[:], in_=beta_in[:, :])

        # per-channel partial sums of y and y^2: col = ot*4 + bi*2 + h
        part_s1 = statp.tile([128, OT * B_LOC * NH], F32)
        part_s2 = statp.tile([128, OT * B_LOC * NH], F32)

        y_tiles = {}  # (bi, ot) -> tile (128, N) f32

        def head_a(bi):
            st = {"bi": bi}
            # load x (C, N) f32
            x_ct = []
            for ct in range(CT):
                t = xload.tile([128, N], F32, tag="x")
                nc.sync.dma_start(
                    out=t[:], in_=x_in[bi, ct * 128:(ct + 1) * 128, :]
                )
                x_ct.append(t)
            if bi == 0:
                load_params()

            # column norms: s_raw[m] = sum_c x[c,m]^2 via ones-matmul
            xsq_ct = []
            for ct in range(CT):
                t = sqp.tile([128, N], F32, tag="xsq")
                nc.scalar.activation(t[:], x_ct[ct][:], AF.Square)
                xsq_ct.append(t)

            rnorm_row = rowp.tile([1, N], F32, tag="rnorm_row")
            for h in range(NH):
                hs = slice(h * 512, (h + 1) * 512)
                ps = ps_row.tile([1, 512], F32, tag="srow")
                for ct in range(CT):
                    nc.tensor.matmul(
                        out=ps[:],
                        lhsT=ones_col[:],
                        rhs=xsq_ct[ct][:, hs],
                        start=(ct == 0),
                        stop=(ct == CT - 1),
                    )
                srt = rowp.tile([1, 512], F32, tag="srt")
                nc.scalar.activation(srt[:], ps[:], AF.Sqrt)
                nc.vector.reciprocal(rnorm_row[:, hs], srt[:])

            # broadcast rnorm to 128 partitions via K=1 PE matmul
            rnorm_bc = bcp.tile([128, N], F32, tag="rnorm_bc")
            for h in range(NH):
                hs = slice(h * 512, (h + 1) * 512)
                ps = ps_bc.tile([128, 512], F32, tag="bc")
                nc.tensor.matmul(
                    out=ps[:],
                    lhsT=ones_row32[:],
                    rhs=rnorm_row[:, hs],
                    start=True,
                    stop=True,
                )
                nc.scalar.copy(rnorm_bc[:, hs], ps[:])

            # normalize: xn32 f32 (Gram + transposes), fp16 cast (conv rhs)
            xn32_ct = []
            xn_ct = []
            for ct in range(CT):
                t32 = xnp.tile([128, N], F32, tag="xn32")
                nc.vector.tensor_tensor(
                    t32[:], x_ct[ct][:], rnorm_bc[:], op=AluOpType.mult
                )
                xn32_ct.append(t32)
            for ct in range(CT):
                t = xnp.tile([128, N], F16, tag="xn")
                nc.scalar.copy(t[:], xn32_ct[ct][:])
                xn_ct.append(t)

            st.update(x_ct=x_ct, xn32_ct=xn32_ct, xn_ct=xn_ct,
                      rnorm_bc=rnorm_bc)
            return st

        def head_b(st):
            bi = st["bi"]
            xn32_ct = st["xn32_ct"]
            # transpose xn32 -> (N, C) fp16 rows (cast in the PSUM evict)
            xn_nc = []
            for nb in range(NB):
                t = xnncp.tile([128, C], F16, tag="xn_nc")
                for ct in range(CT):
                    ps = ps_tp.tile([128, 128], F32, tag="tp")
                    nc.tensor.transpose(
                        out=ps[:],
                        in_=xn32_ct[ct][:, nb * 128:(nb + 1) * 128],
                        identity=ident32[:],
                    )
                    nc.scalar.copy(t[:, ct * 128:(ct + 1) * 128], ps[:])
                nc.sync.dma_start(
                    out=xn_rows[bi][nb * 128:(nb + 1) * 128, :], in_=t[:]
                )
                xn_nc.append(t)

            md_cn = []
            for ct in range(CT):
                md_cn.append(
                    mdcnp.tile([128, N], F16, tag="md_cn", name=f"md_cn{bi}_{ct}")
                )
            st.update(xn_nc=xn_nc, md_cn=md_cn)
            return st

        def topk_part(st, rb, raw=False):
            bi = st["bi"]
            ops = st["x_ct"] if raw else st["xn32_ct"]
            rbs = slice(rb * 128, (rb + 1) * 128)
            score = scorep.tile([128, N], F32, tag="score")
            for h in range(NH):
                hs = slice(h * 512, (h + 1) * 512)
                ps = ps_mm.tile([128, 512], F32, tag="mm")
                for ct in range(CT):
                    nc.tensor.matmul(
                        out=ps[:],
                        lhsT=ops[ct][:, rbs],
                        rhs=ops[ct][:, hs],
                        start=(ct == 0),
                        stop=(ct == CT - 1),
                    )
                if raw:
                    # raw-x Gram column-scaled by rnorm[m]: same per-row
                    # ordering as the normalized Gram (row scale > 0)
                    nc.vector.scalar_tensor_tensor(
                        out=score[:, hs],
                        in0=ps[:],
                        scalar=1.0,
                        in1=st["rnorm_bc"][:, hs],
                        op0=AluOpType.mult,
                        op1=AluOpType.mult,
                    )
                else:
                    # score = G (unit-norm rows: larger G == nearer)
                    nc.scalar.copy(score[:, hs], ps[:])
            # self-exclusion: score[p, rb*128+p] -= BIG
            nc.vector.tensor_tensor(
                score[:, rbs], score[:, rbs], big_i[:], op=AluOpType.subtract
            )

            # top-16: 8 + 8 via max8/max_index(u16)/match_replace
            idx16h = idxp.tile([128, K_G], U16, tag="idxh")
            m8 = idxp.tile([128, 8], F32, tag="m8")
            nc.vector.max(out=m8[:], in_=score[:])
            nc.vector.max_index(
                out=idx16h[:, 0:8], in_max=m8[:], in_values=score[:]
            )
            nc.vector.match_replace(
                out=score[:],
                in_to_replace=m8[:],
                in_values=score[:],
                imm_value=float(-BIG),
            )
            m8b = idxp.tile([128, 8], F32, tag="m8b")
            nc.vector.max(out=m8b[:], in_=score[:])
            nc.vector.max_index(
                out=idx16h[:, 8:16], in_max=m8b[:], in_values=score[:]
            )
            idx32 = idxp.tile([128, K_G], U32, tag="idx32")
            nc.vector.tensor_copy(idx32[:], idx16h[:])
            return idx32

        def gather_part(st, rb, idx32):
            bi = st["bi"]
            # gather 16 neighbor rows (per-k indirect DMA, fp16 rows).
            # ~8ns/descriptor of Q7 SWDGE time is the hard floor here;
            # dma_gather costs the same Q7 time but pipelines worse.
            nbr = nbrp.tile([128, K_G, C], F16, tag="nbr")
            for s in range(K_G):
                nc.gpsimd.indirect_dma_start(
                    out=nbr[:, s, :],
                    out_offset=None,
                    in_=xn_rows[bi][:],
                    in_offset=IndirectOffsetOnAxis(
                        ap=idx32[:, s:s + 1], axis=0
                    ),
                )
            return nbr

        def topk_gather(st, rb):
            return gather_part(st, rb, topk_part(st, rb))

        def trees_md(st, rb, nbr):
            xn_nc = st["xn_nc"]
            md_cn = st["md_cn"]
            rbs = slice(rb * 128, (rb + 1) * 128)
            tmax = treep.tile([128, K_G // 2, C], F16, tag="tmax")
            tmin = treep.tile([128, K_G // 2, C], F16, tag="tmin")
            nc.vector.tensor_tensor(
                tmax[:], nbr[:, 0:8, :], nbr[:, 8:16, :], op=AluOpType.max
            )
            nc.vector.tensor_tensor(
                tmin[:], nbr[:, 0:8, :], nbr[:, 8:16, :], op=AluOpType.min
            )
            w_ = 4
            while w_ >= 1:
                nc.vector.tensor_tensor(
                    tmax[:, 0:w_, :],
                    tmax[:, 0:w_, :],
                    tmax[:, w_:2 * w_, :],
                    op=AluOpType.max,
                )
                nc.vector.tensor_tensor(
                    tmin[:, 0:w_, :],
                    tmin[:, 0:w_, :],
                    tmin[:, w_:2 * w_, :],
                    op=AluOpType.min,
                )
                w_ //= 2

            # md = max(xn - min, max - xn)
            md_nc = mdncp.tile([128, C], F16, tag="md_nc")
            d1 = mdncp.tile([128, C], F16, tag="d1")
            nc.vector.tensor_tensor(
                d1[:], xn_nc[rb][:], tmin[:, 0, :], op=AluOpType.subtract
            )
            nc.vector.tensor_tensor(
                md_nc[:], tmax[:, 0, :], xn_nc[rb][:], op=AluOpType.subtract
            )
            nc.vector.tensor_tensor(
                md_nc[:], md_nc[:], d1[:], op=AluOpType.max
            )

            # transpose md block into (C, N) fp16 tiles
            for ct in range(CT):
                ps = ps_tp.tile([128, 128], F16, tag="tp16")
                nc.tensor.transpose(
                    out=ps[:],
                    in_=md_nc[:, ct * 128:(ct + 1) * 128],
                    identity=ident16[:],
                )
                nc.scalar.copy(md_cn[ct][:, rbs], ps[:])

        def conv_half(st, h):
            # conv for columns [h*512,(h+1)*512) -- only needs md_cn from
            # row blocks h*4..h*4+3, so h=0 can run under the gather stream
            bi = st["bi"]
            xn_ct = st["xn_ct"]
            md_cn = st["md_cn"]
            hs = slice(h * 512, (h + 1) * 512)
            for ot in range(OT):
                ots = slice(ot * 128, (ot + 1) * 128)
                if (bi, ot) not in y_tiles:
                    yt = ypool.tile([128, N], F32, tag="y", name=f"y{bi}_{ot}")
                    y_tiles[(bi, ot)] = yt
                yt = y_tiles[(bi, ot)]
                ps = ps_mm.tile([128, 512], F32, tag="mm")
                for ct in range(CT):
                    nc.tensor.matmul(
                        out=ps[:],
                        lhsT=wev[ct][:, ots],
                        rhs=xn_ct[ct][:, hs],
                        start=(ct == 0),
                        stop=False,
                    )
                for ct in range(CT):
                    nc.tensor.matmul(
                        out=ps[:],
                        lhsT=wod[ct][:, ots],
                        rhs=md_cn[ct][:, hs],
                        start=False,
                        stop=(ct == CT - 1),
                    )
                # move PSUM->SBUF on ACT with fused per-channel sum
                col = ot * (B_LOC * NH) + bi * NH + h
                nc.scalar.activation(
                    yt[:, hs],
                    ps[:],
                    AF.Copy,
                    accum_out=part_s1[:, col:col + 1],
                )
                # sumsq via ACT Square with fused per-channel sum
                sq_scr = ysqp.tile([128, 512], F32, tag="ysq")
                nc.scalar.activation(
                    sq_scr[:],
                    yt[:, hs],
                    AF.Square,
                    accum_out=part_s2[:, col:col + 1],
                )

        # batch-0 head (its first Gram/topk chains overlap its own
        # transposes), batch-1 head under batch-0 gathers, interleaved row
        # blocks; convs at the very end so they don't block the last top-ks.
        from collections import deque

        states = [None, None]
        states[0] = head_a(0)
        order = [(0, 0), (0, 1)]
        tail0 = [(0, r) for r in range(2, NB)]
        all1 = [(1, r) for r in range(NB)]
        for i in range(len(all1)):
            order.append(all1[i])
            if i < len(tail0):
                order.append(tail0[i])
        i00 = topk_part(states[0], 0, raw=True)
        i01 = topk_part(states[0], 1, raw=True)
        head_b(states[0])
        pending = {0: deque(), 1: deque()}
        pending[0].append((0, gather_part(states[0], 0, i00)))
        pending[0].append((1, gather_part(states[0], 1, i01)))
        trees_done = {0: 0, 1: 0}

        def pop_tree(bj):
            trees_md(states[bj], *pending[bj].popleft())
            trees_done[bj] += 1
            if trees_done[bj] == 4:
                conv_half(states[bj], 0)

        for bi, rb in order[2:]:
            if states[bi] is None:
                states[bi] = head_a(bi)
                head_b(states[bi])
            nbr = topk_gather(states[bi], rb)
            pending[bi].append((rb, nbr))
            if len(pending[bi]) > 3:
                pop_tree(bi)
        for bi in range(B_LOC):
            while pending[bi]:
                pop_tree(bi)
            conv_half(states[bi], 1)

        # ---- BN stats: reduce partials, all-reduce across cores ----
        stats_sb = statp.tile([128, 2 * OT], F32)
        nc.vector.tensor_reduce(
            stats_sb[:, 0:OT],
            part_s1[:].rearrange("p (o q) -> p o q", q=B_LOC * NH),
            axis=AX,
            op=AluOpType.add,
        )
        nc.vector.tensor_reduce(
            stats_sb[:, OT:2 * OT],
            part_s2[:].rearrange("p (o q) -> p o q", q=B_LOC * NH),
            axis=AX,
            op=AluOpType.add,
        )
        nc.sync.dma_start(out=stats_in[:, :], in_=stats_sb[:])
        nc.gpsimd.collective_compute(
            "AllReduce",
            AluOpType.add,
            replica_groups=[list(range(N_CORES))],
            ins=[stats_in.ap().opt()],
            outs=[stats_out.ap().opt()],
        )
        stats_red = statp.tile([128, 2 * OT], F32)
        nc.sync.dma_start(out=stats_red[:], in_=stats_out[:, :])

        # mean/var/affine (per channel; channel c = partition p, col ot)
        inv_cnt = 1.0 / float(B * N)
        mean4 = statp.tile([128, OT], F32)
        nc.vector.tensor_scalar_mul(mean4[:], stats_red[:, 0:OT], inv_cnt)
        msq = statp.tile([128, OT], F32)
        nc.vector.tensor_tensor(msq[:], mean4[:], mean4[:], op=AluOpType.mult)
        var4 = statp.tile([128, OT], F32)
        nc.vector.scalar_tensor_tensor(
            out=var4[:],
            in0=stats_red[:, OT:2 * OT],
            scalar=inv_cnt,
            in1=msq[:],
            op0=AluOpType.mult,
            op1=AluOpType.subtract,
        )
        nc.vector.tensor_scalar_add(var4[:], var4[:], float(BN_EPS))
        std4 = statp.tile([128, OT], F32)
        nc.scalar.activation(std4[:], var4[:], AF.Sqrt)
        rstd4 = statp.tile([128, OT], F32)
        nc.vector.reciprocal(rstd4[:], std4[:])
        a4 = statp.tile([128, OT], F32)
        nc.vector.tensor_tensor(a4[:], gamma4[:], rstd4[:], op=AluOpType.mult)
        b4 = statp.tile([128, OT], F32)
        nc.vector.scalar_tensor_tensor(
            out=b4[:],
            in0=mean4[:],
            scalar=-1.0,
            in1=a4[:],
            op0=AluOpType.mult,
            op1=AluOpType.mult,
        )
        nc.vector.tensor_tensor(b4[:], b4[:], beta4[:], op=AluOpType.add)

        # ---- fused BN + exact gelu on ACT, then store ----
        for bi in range(B_LOC):
            for ot in range(OT):
                yt = y_tiles[(bi, ot)]
                for h in range(NH):
                    hs = slice(h * 512, (h + 1) * 512)
                    nc.scalar.activation(
                        yt[:, hs],
                        yt[:, hs],
                        AF.Gelu if use_gelu else AF.Copy,
                        bias=b4[:, ot:ot + 1] if use_gelu else 0.0,
                        scale=a4[:, ot:ot + 1],
                    )
                nc.sync.dma_start(
                    out=out_dram[bi, ot * 128:(ot + 1) * 128, :], in_=yt[:]
                )

    nc.compile()
    return nc


_NC_CACHE = None


def _get_nc():
    global _NC_CACHE
    if _NC_CACHE is None:
        _NC_CACHE = build_kernel()
    return _NC_CACHE


def _prep_shared(w, gamma, beta):
    w = np.asarray(w, np.float32)
    wev = np.ascontiguousarray(w[:, 0::2].T).astype(np.float16)  # (C, OUT)
    wod = np.ascontiguousarray(w[:, 1::2].T).astype(np.float16)
    gamma4 = np.ascontiguousarray(
        np.asarray(gamma, np.float32).reshape(OT, 128).T
    )
    beta4 = np.ascontiguousarray(np.asarray(beta, np.float32).reshape(OT, 128).T)
    return wev, wod, gamma4, beta4


def kernel(x, w, b, gamma, beta):
    x = np.ascontiguousarray(np.asarray(x, np.float32))
    assert x.shape == (B, C, N), x.shape
    wev, wod, gamma4, beta4 = _prep_shared(w, gamma, beta)
    # b cancels exactly in training-mode BN (see module docstring).
    nc = _get_nc()
    in_maps = [
        {
            "x": np.ascontiguousarray(x[c * B_LOC:(c + 1) * B_LOC]),
            "wev": wev,
            "wod": wod,
            "gamma4": gamma4,
            "beta4": beta4,
        }
        for c in range(N_CORES)
    ]
    res = run_bass_kernel_spmd(nc, in_maps, core_ids=list(range(N_CORES)))
    out = np.concatenate([res.results[c]["out"] for c in range(N_CORES)], axis=0)
    return out[..., None].astype(np.float32)
